# revision 1
# baseline (speedup 1.0000x reference)
"""Multi-head attention (B=8, S=1024, D=1024, H=16) on 8 TRN2 NeuronCores.

Sharding: pure data-parallel over batch — core b computes batch b entirely
locally (no collectives). All matmuls run in bf16 with fp32 PSUM accumulation.

Per-core dataflow (host pre-transposes inputs/weights so no on-chip input
transposes are needed):
  Q_t[d,s], K_t[d,s] projected per d-tile (scale 1/sqrt(dk) folded into
  WQ/bq on the host; WQ/WK arrive as host-prearranged column slabs so they
  stream through SBUF instead of staying resident). V[s,d] is stored with a
  ones-column interleaved per head so the attention-value matmul also
  produces softmax row sums. Per head:
    S.T[k,q] = K_t_h.T @ Q_t_h   (single K=64 matmul per 512-chunk)
    E.T = exp(S.T) * mask.T      (ACT exp from PSUM; mask mult on GPSIMD)
    psum[q, 0:65] = sum_k E.T_tile.T @ [V_h | 1]  -> out + rowsum
    attnout[q, d_h] = psum[:,0:64] * recip(psum[:,64])   (DVE)
  attnout transposed via PE -> WO projection -> + bias -> out[s,o] fp32.

Schedule (engine streams are static, so emission order is the scheduler):
V projection first — it is ~2x denser PE work per DMA byte, filling the
wire-paced startup, with its psum groups 3-way interleaved so each arriving
wv/xv tile feeds 3x the matmuls. Q/K projections run two head-pairs ahead
as PE filler under the ACT exp latency (exp is ACT's ~133us floor); masks
run on GPSIMD so the DVE queue services projection evictions (which gate
the next pair's QK) immediately; the tail software-pipelines the last
head's AV with its transposes and the WO s-tiles (AV[j] / transp[j-1] /
WO[j-2]). Cost-model time: ~247us/core at 80% PE occupancy.
"""

import os
import sys
from contextlib import ExitStack

import numpy as np

if os.environ.get("JAX_PLATFORMS") == "cpu":
    # bass execution needs the neuron/axon jax backend
    del os.environ["JAX_PLATFORMS"]

for _p in ("/opt/trn_rl_repo",):
    if _p not in sys.path and os.path.isdir(_p):
        sys.path.insert(0, _p)

import ml_dtypes

import concourse.bass as bass
import concourse.mybir as mybir
import concourse.tile as tile
from concourse import bacc
from concourse.bass import ds, ts
from concourse.bass_utils import run_bass_kernel_spmd
from concourse.masks import make_identity

BF16 = mybir.dt.bfloat16
F32 = mybir.dt.float32
NPBF = ml_dtypes.bfloat16

B, S, D, H, DK = 8, 1024, 1024, 16, 64
P = 128
NT = D // P  # 8 tiles along any 1024 dim
CH = 512  # matmul moving-dim chunk (one PSUM bank of fp32)
NCH = S // CH  # 2

MASK_ON_GPSIMD = True

LAST_RESULTS = None
_NC_CACHE = None


def build_nc():
    nc = bacc.Bacc("TRN2", target_bir_lowering=False, debug=False)

    xq = nc.dram_tensor("xq", [D, S], BF16, kind="ExternalInput")  # q[b].T
    xk = nc.dram_tensor("xk", [D, S], BF16, kind="ExternalInput")
    xv = nc.dram_tensor("xv", [D, S], BF16, kind="ExternalInput")
    # wq/wk: host-prearranged column slabs [t][p][i*128+f] = W.T[i*128+p, t*128+f]
    wq = nc.dram_tensor("wq", [NT, P, D], BF16, kind="ExternalInput")
    wk = nc.dram_tensor("wk", [NT, P, D], BF16, kind="ExternalInput")
    wv = nc.dram_tensor("wv", [D, D], BF16, kind="ExternalInput")  # WV_w.T
    wo = nc.dram_tensor("wo", [D, D], BF16, kind="ExternalInput")  # WO_w.T
    bq = nc.dram_tensor("bq", [P, NT], F32, kind="ExternalInput")  # WQ_b/8
    bk = nc.dram_tensor("bk", [P, NT], F32, kind="ExternalInput")
    bvb = nc.dram_tensor("bvb", [P, H * 65], BF16, kind="ExternalInput")
    bob = nc.dram_tensor("bob", [P, D], F32, kind="ExternalInput")
    mt = nc.dram_tensor("mt", [S, S], BF16, kind="ExternalInput")  # mask[b,0].T
    out = nc.dram_tensor("out", [S, D], F32, kind="ExternalOutput")

    with tile.TileContext(nc) as tc, ExitStack() as ctx:
        pers = ctx.enter_context(tc.tile_pool(name="pers", bufs=1))
        # xq+xk resident for the whole kernel
        xld = ctx.enter_context(tc.tile_pool(name="xld", bufs=16))
        xvp = ctx.enter_context(tc.tile_pool(name="xvp", bufs=8))
        # wv (early) then wo (late) share 8 slots
        wld = ctx.enter_context(tc.tile_pool(name="wld", bufs=8))
        wslab = ctx.enter_context(tc.tile_pool(name="wslab", bufs=4))
        # q/k projection outputs: only live for their head pair -> rotate
        qkp = ctx.enter_context(tc.tile_pool(name="qkp", bufs=3))
        epool = ctx.enter_context(tc.tile_pool(name="epool", bufs=16))
        aop = ctx.enter_context(tc.tile_pool(name="aop", bufs=16))
        opool = ctx.enter_context(tc.tile_pool(name="opool", bufs=2))
        rpool = ctx.enter_context(tc.tile_pool(name="rpool", bufs=8))
        # one 2-bank psum pool shared by projections / scores / WO ...
        pp = ctx.enter_context(tc.tile_pool(name="pp", bufs=3, space="PSUM"))
        # ... and a 1-bank pool for AV accumulation + attnout transposes
        ps_av = ctx.enter_context(tc.tile_pool(name="ps_av", bufs=2, space="PSUM"))

        # ---- persistent tiles ----
        vv = [
            pers.tile([P, H * 65], BF16, name=f"vv{t}", tag=f"vv{t}")
            for t in range(NT)
        ]
        msk = [pers.tile([P, S], BF16, name=f"mk{t}", tag=f"mk{t}") for t in range(NT)]
        aot = [pers.tile([P, S], BF16, name=f"at{t}", tag=f"at{t}") for t in range(NT)]
        ident = pers.tile([P, P], BF16, name="ident", tag="ident")
        bq_sb = pers.tile([P, NT], F32, name="bq_sb", tag="bq_sb")
        bk_sb = pers.tile([P, NT], F32, name="bk_sb", tag="bk_sb")
        bv_sb = pers.tile([P, H * 65], BF16, name="bv_sb", tag="bv_sb")
        bo_sb = pers.tile([P, D], F32, name="bo_sb", tag="bo_sb")

        make_identity(nc, ident)
        nc.sync.dma_start(bq_sb[:], bq[:])
        nc.sync.dma_start(bk_sb[:], bk[:])

        def load_slab(wdram, ot):
            wsl = wslab.tile([P, D], BF16, name="wsl", tag="ws")
            nc.sync.dma_start(wsl[:], wdram[ot])
            return wsl

        # ---- input DMAs: V-path first — V-proj is ~2x denser PE work per DMA
        # byte than the Q/K path, so it best fills the DMA-paced startup.
        # x/slab/mask DMAs land while V-proj computes. ----
        wvsb = []
        xvsb = []
        for i in range(NT):
            w_t = wld.tile([P, D], BF16, name=f"wv{i}", tag="w")
            nc.sync.dma_start(w_t[:], wv[ts(i, P), :])
            wvsb.append(w_t)
            x_t = xvp.tile([P, S], BF16, name=f"xv{i}", tag="xv")
            nc.sync.dma_start(x_t[:], xv[ts(i, P), :])
            xvsb.append(x_t)
            if i == 0:
                nc.sync.dma_start(bv_sb[:], bvb[:])
        sl_q = load_slab(wq, 0)
        sl_k = load_slab(wk, 0)
        xqsb, xksb = [], []
        for i in range(NT):
            x_t = xld.tile([P, S], BF16, name=f"xq{i}", tag="x")
            nc.sync.dma_start(x_t[:], xq[ts(i, P), :])
            xqsb.append(x_t)
            x_t = xld.tile([P, S], BF16, name=f"xk{i}", tag="x")
            nc.sync.dma_start(x_t[:], xk[ts(i, P), :])
            xksb.append(x_t)
        for i in range(NT):
            nc.sync.dma_start(msk[i][:], mt[ts(i, P), :])
        nc.sync.dma_start(bo_sb[:], bob[:])

        def project(wsl, bias, ot, xtiles, pname):
            ps = pp.tile([P, S], F32, name="ps_pj", tag="pp")
            for c in range(NCH):
                for i in range(NT):
                    nc.tensor.matmul(
                        ps[:, ts(c, CH)],
                        wsl[:, ts(i, P)],
                        xtiles[i][:, ts(c, CH)],
                        start=(i == 0),
                        stop=(i == NT - 1),
                    )
            dst = qkp.tile([P, S], BF16, name=pname, tag=pname[0])
            # two chunk evictions so the next pair's first QK matmuls (which
            # read chunk 0) unblock ~0.5us earlier
            for c in range(NCH):
                nc.vector.tensor_scalar_add(
                    dst[:, ts(c, CH)], ps[:, ts(c, CH)], bias[:, ds(ot, 1)]
                )
            return dst

        def head_qk(h, qt_t, kt_t):
            """scores -> exp -> mask for head h; returns the 8 E.T tiles."""
            prow = (h % 2) * 64
            eh = []
            for i in range(NT):
                st_ps = pp.tile([P, S], F32, name="st", tag="pp")
                for c in range(NCH):
                    nc.tensor.matmul(
                        st_ps[:, ts(c, CH)],
                        kt_t[ds(prow, 64), ts(i, P)],
                        qt_t[ds(prow, 64), ts(c, CH)],
                        start=True,
                        stop=True,
                    )
                e = epool.tile([P, S], BF16, name=f"e{i}", tag="e")
                nc.scalar.activation(e[:], st_ps[:], mybir.ActivationFunctionType.Exp)
                # masks all on GPSIMD: keeps the DVE queue free of ACT-paced
                # work so the q/k projection evictions (which gate the next
                # pair's QK) run as soon as their psum completes
                if MASK_ON_GPSIMD:
                    nc.gpsimd.tensor_mul(e[:], e[:], msk[i][:])
                else:
                    nc.vector.tensor_mul(e[:], e[:], msk[i][:])
                eh.append(e)
            return eh

        def head_av(h, eh, aopair):
            prow = (h % 2) * 64
            for j in range(NT):
                av = ps_av.tile([P, P], F32, name="av", tag="av")
                for i in range(NT):
                    nc.tensor.matmul(
                        av[:, 0:65],
                        eh[i][:, ts(j, P)],
                        vv[i][:, ds(h * 65, 65)],
                        start=(i == 0),
                        stop=(i == NT - 1),
                    )
                rc = rpool.tile([P, 1], F32, name="rc", tag="rc")
                nc.vector.reciprocal(rc[:], av[:, ds(64, 1)])
                nc.vector.tensor_scalar_mul(
                    aopair[j][:, ds(prow, 64)], av[:, 0:64], rc[:]
                )

        def transpose_pair(t, aopair):
            # all 8 [128,128]bf16 transposes fit ONE psum bank: 1 slot + 1 big
            # DVE copy instead of 8 of each — the next AV's psum slot frees
            # much sooner. j=0's start=True clears the bank's has_written bits
            # (stale from the slot's previous user); j>0 then overwrite their
            # untouched ranges.
            ptb = ps_av.tile([P, S], BF16, name="ptb", tag="av")
            for j in range(NT):
                nc.tensor.matmul(
                    ptb[:, ts(j, P)],
                    aopair[j][:],
                    ident[:],
                    is_transpose=True,
                    start=(j == 0),
                    stop=(j == NT - 1),
                    skip_group_check=True,
                )
            nc.vector.tensor_copy(aot[t][:], ptb[:])

        # ---- V projection first (dense PE work during input DMA; AV depends
        # on all of V) ----
        # V[s, d]: stationary = x.T [i,s]-tile, moving = W.T [i,o].
        # Groups are processed 3 at a time (= pp bufs) with their i-loops
        # interleaved: the stream is paced by each wv/xv tile's DMA arrival,
        # so 3-way interleave gives PE 3x the work per arriving tile.
        # 4 groups per batch: 3 through pp (2-bank tiles) + 1 split into its
        # two 512-chunks through the ps_av slots (1 bank each, idle during
        # V-proj) -> 8 matmuls per arriving wv/xv tile instead of 6 during
        # the DMA-paced phase, and 2 batches instead of 3.
        for g0 in (0, 4):
            batch = list(range(g0, g0 + 4))
            pss = {}
            for st_ in batch:
                nc.gpsimd.memset(
                    vv[st_].rearrange("p (g c) -> p g c", c=65)[:, :, 64:65], 1.0
                )
            for st_ in batch[:3]:
                pss[st_] = pp.tile([P, D], F32, name="ps_pv", tag="pp")
            sp = batch[3]
            half = [
                ps_av.tile([P, CH], F32, name="ps_ph", tag="av") for _ in range(NCH)
            ]
            for i in range(NT):
                for st_ in batch[:3]:
                    for c in range(NCH):
                        nc.tensor.matmul(
                            pss[st_][:, ts(c, CH)],
                            xvsb[i][:, ts(st_, P)],
                            wvsb[i][:, ts(c, CH)],
                            start=(i == 0),
                            stop=(i == NT - 1),
                        )
                for c in range(NCH):
                    nc.tensor.matmul(
                        half[c][:],
                        xvsb[i][:, ts(sp, P)],
                        wvsb[i][:, ts(c, CH)],
                        start=(i == 0),
                        stop=(i == NT - 1),
                    )
            for st_ in batch[:3]:
                # scatter 16 head-blocks of 64 into 65-strided layout, + bias
                nc.vector.tensor_add(
                    vv[st_].rearrange("p (g c) -> p g c", c=65)[:, :, 0:64],
                    pss[st_].rearrange("p (g c) -> p g c", c=64),
                    bv_sb.rearrange("p (g c) -> p g c", c=65)[:, :, 0:64],
                )
            for c in range(NCH):
                g0c = c * 8
                nc.vector.tensor_add(
                    vv[sp][:, ds(g0c * 65, 8 * 65)].rearrange(
                        "p (g c) -> p g c", c=65
                    )[:, :, 0:64],
                    half[c].rearrange("p (g c) -> p g c", c=64),
                    bv_sb[:, ds(g0c * 65, 8 * 65)].rearrange(
                        "p (g c) -> p g c", c=65
                    )[:, :, 0:64],
                )

        # ---- main loop over head pairs ----
        # static PE order per pair: QK (feeds ACT) -> next-pair projection
        # (fills PE while ACT runs the exps) -> AV(2t) -> previous pair's
        # transposes (extra PE filler before AV(2t+1)'s E is ready) -> AV(2t+1)
        def wo_stile(j, wosb):
            ps = pp.tile([P, D], F32, name="ps_wo", tag="pp")
            for c in range(NCH):
                for i in range(NT):
                    nc.tensor.matmul(
                        ps[:, ts(c, CH)],
                        aot[i][:, ts(j, P)],
                        wosb[i][:, ts(c, CH)],
                        start=(i == 0),
                        stop=(i == NT - 1),
                    )
            osb = opool.tile([P, D], F32, name="osb", tag="osb")
            nc.vector.tensor_add(osb[:], ps[:], bo_sb[:])
            nc.sync.dma_start(out[ts(j, P), :], osb[:])

        # projections run two pairs ahead (qkp bufs=3 per tag: current, next,
        # next-next) so iteration 0's AVs have a full exp-latency of PE filler
        qts = {0: project(sl_q, bq_sb, 0, xqsb, "qt")}
        kts = {0: project(sl_k, bk_sb, 0, xksb, "kt")}
        qts[1] = project(load_slab(wq, 1), bq_sb, 1, xqsb, "qt")
        kts[1] = project(load_slab(wk, 1), bk_sb, 1, xksb, "kt")

        prev = None
        wosb = []
        for t in range(NT):
            qt_t, kt_t = qts.pop(t), kts.pop(t)
            aopair = [
                aop.tile([P, P], BF16, name=f"aop{j}", tag="aop") for j in range(NT)
            ]
            eh_a = head_qk(2 * t, qt_t, kt_t)
            eh_b = head_qk(2 * t + 1, qt_t, kt_t)
            if t == 5:
                # prefetch WO weights (reuses the wv slots, long since free)
                for i in range(NT):
                    w_t = wld.tile([P, D], BF16, name=f"wo{i}", tag="w")
                    nc.sync.dma_start(w_t[:], wo[ts(i, P), :])
                    wosb.append(w_t)
            if t + 2 < NT:
                qts[t + 2] = project(load_slab(wq, t + 2), bq_sb, t + 2, xqsb, "qt")
                kts[t + 2] = project(load_slab(wk, t + 2), bk_sb, t + 2, xksb, "kt")
            if prev is not None:
                transpose_pair(t - 1, prev)
            head_av(2 * t, eh_a, aopair)
            if t < NT - 1:
                head_av(2 * t + 1, eh_b, aopair)
            else:
                # tail: interleave the last head's AV with its transposes and
                # the WO s-tiles so the output projection starts per-j instead
                # of waiting for the whole pair
                prow = 64
                for j in range(NT + 2):
                    if j < NT:
                        av = ps_av.tile([P, P], F32, name="av", tag="av")
                        for i in range(NT):
                            nc.tensor.matmul(
                                av[:, 0:65],
                                eh_b[i][:, ts(j, P)],
                                vv[i][:, ds((2 * t + 1) * 65, 65)],
                                start=(i == 0),
                                stop=(i == NT - 1),
                            )
                        rc = rpool.tile([P, 1], F32, name="rc", tag="rc")
                        nc.vector.reciprocal(rc[:], av[:, ds(64, 1)])
                        nc.vector.tensor_scalar_mul(
                            aopair[j][:, ds(prow, 64)], av[:, 0:64], rc[:]
                        )
                    if 1 <= j <= NT:
                        pt = ps_av.tile([P, P], BF16, name="pt", tag="av")
                        nc.tensor.transpose(pt[:], aopair[j - 1][:], ident[:])
                        nc.vector.tensor_copy(aot[t][:, ts(j - 1, P)], pt[:])
                    if j >= 2:
                        wo_stile(j - 2, wosb)
            prev = aopair

    nc.compile()
    return nc


def prep_inputs(q, k, v, mask, WQ_w, WQ_b, WK_w, WK_b, WV_w, WV_b, WO_w, WO_b):
    """Build the 8 per-core input maps (host-side layout prep)."""
    f32 = np.float32

    def slabs(wt):  # [D,D] W.T -> [NT, P, D]: [t][p][i*128+f] = wt[i*128+p, t*128+f]
        return np.ascontiguousarray(
            wt.reshape(NT, P, NT, P).transpose(2, 1, 0, 3).reshape(NT, P, D)
        )

    wq_t = slabs((WQ_w.astype(f32) * 0.125).T).astype(NPBF)
    wk_t = slabs(WK_w.astype(f32).T).astype(NPBF)
    wv_t = np.ascontiguousarray(WV_w.astype(f32).T).astype(NPBF)
    wo_t = np.ascontiguousarray(WO_w.astype(f32).T).astype(NPBF)
    bq_l = np.ascontiguousarray((WQ_b.astype(f32) * 0.125).reshape(NT, P).T)
    bk_l = np.ascontiguousarray(WK_b.astype(f32).reshape(NT, P).T)
    bvb = np.zeros((P, H * 65), NPBF)
    bv_f = WV_b.astype(f32)
    for h in range(H):
        bvb[:, h * 65 : h * 65 + 64] = bv_f[h * 64 : (h + 1) * 64].astype(NPBF)[None, :]
    bob = np.ascontiguousarray(np.broadcast_to(WO_b.astype(f32), (P, D)))

    in_maps = []
    for b in range(B):
        in_maps.append(
            {
                "xq": np.ascontiguousarray(q[b].astype(f32).T).astype(NPBF),
                "xk": np.ascontiguousarray(k[b].astype(f32).T).astype(NPBF),
                "xv": np.ascontiguousarray(v[b].astype(f32).T).astype(NPBF),
                "wq": wq_t,
                "wk": wk_t,
                "wv": wv_t,
                "wo": wo_t,
                "bq": bq_l,
                "bk": bk_l,
                "bvb": bvb,
                "bob": bob,
                "mt": np.ascontiguousarray(mask[b, 0].T.astype(f32)).astype(NPBF),
            }
        )
    return in_maps


def _ensure_neuron_backend():
    # if jax was already initialized cpu-only (e.g. JAX_PLATFORMS=cpu was set
    # before this module was imported), re-discover the neuron/axon backend
    import jax

    try:
        if all(d.platform == "cpu" for d in jax.devices()):
            jax.clear_backends()
    except Exception:
        pass


def kernel(q, k, v, mask, WQ_w, WQ_b, WK_w, WK_b, WV_w, WV_b, WO_w, WO_b):
    global _NC_CACHE, LAST_RESULTS
    _ensure_neuron_backend()
    if _NC_CACHE is None:
        _NC_CACHE = build_nc()
    nc = _NC_CACHE
    in_maps = prep_inputs(
        q, k, v, mask, WQ_w, WQ_b, WK_w, WK_b, WV_w, WV_b, WO_w, WO_b
    )
    res = run_bass_kernel_spmd(nc, in_maps, core_ids=list(range(B)))
    LAST_RESULTS = res
    return np.stack([res.results[b]["out"] for b in range(B)], axis=0).astype(
        np.float32
    )

